# revision 1
# baseline (speedup 1.0000x reference)
"""AFM (Attentional Factorization Machine) Trainium2 kernel.

Model (per sample, F=40 fields, K=64 dim, A=64 attn):
  E = V_table[x]                                  [F, K]
  pw[i,j,:] = E[i,:] * E[j,:]                     [F, F, K]
  h = relu(pw @ W1 + b1); s = h @ W2 + b2         [F, F, 1]
  P = softmax_j(s)                                (b2 cancels)
  out = sigmoid(sum_f lin[x] + sum_{i,k} E[i,k] * (P@E)[i,k] * Wf[k] + bf)

Key identity: interaction = sum_{i,j} P[i,j] * M[i,j] with
  M[i,j] = sum_k pw[i,j,k] * Wf[k],
so only three TensorE passes over pair columns are needed (W1, W2, Wf),
plus a wide sample-parallel softmax backend.

Sharding: pure data-parallel over batch, 256 samples per core, tables and
weights replicated. No collectives.

Per core: 2 macro-groups x 128 samples; each macro = 64 groups of 2 samples
packed on partitions (k of sample A on partitions 0:64, B on 64:128).
Embedding rows are fetched via indirect DMA (one row per partition, 80 rows
per group) from a host-augmented table [V | lin | pad] so the linear-term
values ride along; they are summed per sample by a ones-stationary matmul
accumulated in PSUM. Pair columns are processed in two halves of 800 so the
scores/M PSUM accumulators fit in banks.
"""

import os
import sys

import numpy as np

sys.path.insert(0, "/opt/trn_rl_repo")

B, F, K, A, V = 2048, 40, 64, 64, 100000
NCORES = 8
BL = B // NCORES          # 256 samples per core
NMACRO = 2                # macro-groups per core
MS = 128                  # samples per macro
NG = MS // 2              # 64 groups of 2 samples per macro
NPAIR = F * F             # 1600
HALF = NPAIR // 2         # 800 pair-cols per half-pass
CHUNKS = ((0, 512), (512, 328))   # bank-aligned matmul chunks of the 840 cols
NDIAG = 21                # diagonals d=0..20 (circulant half-pair storage)
NPC = NDIAG * F           # 840 pair columns per group
EXT = 60                  # wrap-extended field axis
VROW = K + 2              # augmented table row: 64 embed + 1 lin + 1 pad
# memory-order diagonal blocks: odd d first (alignment), then even d
DBLOCKS = [2 * t + 1 for t in range(10)] + [2 * t for t in range(11)]

_CACHE = {}


def _build_program():
    from concourse import bass, mybir, tile, bacc
    from contextlib import ExitStack

    f32 = mybir.dt.float32
    bf16 = mybir.dt.bfloat16
    i32 = mybir.dt.int32
    AF = mybir.ActivationFunctionType
    ALU = mybir.AluOpType
    AX = mybir.AxisListType

    nc = bacc.Bacc(
        "TRN2", target_bir_lowering=False, debug=False, num_devices=NCORES
    )

    xT2_p = nc.dram_tensor("xT2", [2 * F, NMACRO * NG], i32, kind="ExternalInput").ap()
    va_p = nc.dram_tensor("va", [V, VROW], f32, kind="ExternalInput").ap()
    ww1_p = nc.dram_tensor("ww1", [128, 128], f32, kind="ExternalInput").ap()
    w2b_p = nc.dram_tensor("w2b", [128, 512], f32, kind="ExternalInput").ap()
    wfb_p = nc.dram_tensor("wfb", [128, 512], f32, kind="ExternalInput").ap()
    ones_p = nc.dram_tensor("onesb", [2 * F, 512], f32, kind="ExternalInput").ap()
    b1v_p = nc.dram_tensor("b1v", [128, 1], f32, kind="ExternalInput").ap()
    bfn_p = nc.dram_tensor("bfn", [128, 1], f32, kind="ExternalInput").ap()
    ident_p = nc.dram_tensor("ident", [128, 128], f32, kind="ExternalInput").ap()
    out_p = nc.dram_tensor("out", [BL], f32, kind="ExternalOutput").ap()

    def win(t, nd, dstride, base=0):
        # sliding-window AP: [128, nd, 40] reading cols base + dstride*t + i
        a = t[:]
        ap0 = list(a.ap)
        return bass.AP(a.tensor, a.offset + base, [list(ap0[0]), [dstride, nd], [1, F]])

    def winT(t, nd, base=0):
        # [128, 40, nd] view of nd stacked 40-col blocks, block axis innermost
        a = t[:]
        ap0 = list(a.ap)
        return bass.AP(a.tensor, a.offset + base, [list(ap0[0]), [1, F], [F, nd]])

    with tile.TileContext(nc) as tc, ExitStack() as top:
        const = top.enter_context(tc.tile_pool(name="const", bufs=1))

        xT2_sb = const.tile([2 * F, NMACRO * NG], i32)
        nc.sync.dma_start(out=xT2_sb[:], in_=xT2_p[:])
        ident_sb = const.tile([128, 128], f32)
        nc.sync.dma_start(out=ident_sb[:], in_=ident_p[:])
        ones_sb = const.tile([2 * F, 512], f32)
        nc.sync.dma_start(out=ones_sb[:], in_=ones_p[:])
        b1v_sb = const.tile([128, 1], f32)
        nc.sync.dma_start(out=b1v_sb[:], in_=b1v_p[:])
        bfn_sb = const.tile([128, 1], f32)
        nc.sync.dma_start(out=bfn_sb[:], in_=bfn_p[:])
        ww1_sb = const.tile([128, 128], bf16)
        nc.gpsimd.dma_start(out=ww1_sb[:], in_=ww1_p[:])
        w2b_sb = const.tile([128, 512], bf16)
        nc.gpsimd.dma_start(out=w2b_sb[:], in_=w2b_p[:])
        wfb_sb = const.tile([128, 512], bf16)
        nc.gpsimd.dma_start(out=wfb_sb[:], in_=wfb_p[:])

        e2_pool = top.enter_context(tc.tile_pool(name="e2", bufs=2))
        gr_pool = top.enter_context(tc.tile_pool(name="gr", bufs=8))
        grb_pool = top.enter_context(tc.tile_pool(name="grb", bufs=4))
        tpt_pool = top.enter_context(tc.tile_pool(name="tpt", bufs=4))
        eeo_pool = top.enter_context(tc.tile_pool(name="eeo", bufs=3))
        pw_pool = top.enter_context(tc.tile_pool(name="pw", bufs=5))
        rh_pool = top.enter_context(tc.tile_pool(name="rh", bufs=5))
        big_pool = top.enter_context(tc.tile_pool(name="big", bufs=2))
        small_pool = top.enter_context(tc.tile_pool(name="small", bufs=4))
        acc_pool = top.enter_context(tc.tile_pool(name="acc", bufs=1, space="PSUM"))
        h_pool = top.enter_context(tc.tile_pool(name="hps", bufs=2, space="PSUM"))
        scps = acc_pool.tile([128, 1024], f32, space="PSUM", tag="sc")
        mvps = acc_pool.tile([128, 1024], f32, space="PSUM", tag="mv")
        # linear-sum accumulator lives in scps' bank-padding tail
        linps = scps[:, 1023:1024]

        # process groups block-interleaved so the narrow-stationary W2/Wf/lin
        # matmuls of consecutive groups land on different PE column groups
        gorder = [(gi % 4) * 16 + gi // 4 for gi in range(NG)]

        for m in range(NMACRO):
            e2_all = e2_pool.tile([128, NG * EXT], bf16, tag="e2all")
            linsum = small_pool.tile([MS, 1], f32, tag="linsum")
            exps = big_pool.tile([128, NPC], f32, tag="exps")
            mvals = big_pool.tile([128, NPC], f32, tag="mvals")

            for gi in range(NG):
                g = gorder[gi]
                b = g // 16
                r = g % 16
                hps = h_pool.tile([128, 1024], f32, space="PSUM", tag="h")
                # ---- gather + transpose + extended-E for this group ----
                gr = gr_pool.tile([2 * F, VROW], f32, tag="gr")
                nc.gpsimd.indirect_dma_start(
                    out=gr[:],
                    out_offset=None,
                    in_=va_p[:],
                    in_offset=bass.IndirectOffsetOnAxis(
                        ap=xT2_sb[:, m * NG + g : m * NG + g + 1], axis=0
                    ),
                )
                tp = hps[0:K, 944:1024]
                nc.tensor.transpose(
                    out=tp, in_=gr[:, 0:K], identity=ident_sb[0 : 2 * F, 0 : 2 * F]
                )
                ee = e2_all[:, g * EXT : (g + 1) * EXT]
                nc.vector.tensor_copy(out=ee[0:K, 0:F], in_=tp[:, 0:F])
                nc.vector.tensor_copy(out=ee[K:128, 0:F], in_=tp[:, F : 2 * F])
                nc.vector.tensor_copy(out=ee[0:K, F:EXT], in_=tp[:, 0 : EXT - F])
                nc.vector.tensor_copy(
                    out=ee[K:128, F:EXT], in_=tp[:, F : F + EXT - F]
                )
                eeo = eeo_pool.tile([128, EXT], bf16, tag="eeo")
                nc.vector.tensor_copy(out=eeo[:, 0 : EXT - 1], in_=ee[:, 1:EXT])
                # ---- pairwise products, circulant (d, i) layout ----
                pw = pw_pool.tile([128, NPC], bf16, tag="pw")
                nc.vector.tensor_tensor(
                    out=pw[:, 0:400].rearrange("p (a b) -> p a b", a=10),
                    in0=ee[:, 0:F].unsqueeze(1).to_broadcast([128, 10, F]),
                    in1=win(eeo, 10, 2),
                    op=ALU.mult,
                )
                nc.vector.tensor_tensor(
                    out=pw[:, 400:NPC].rearrange("p (a b) -> p a b", a=11),
                    in0=ee[:, 0:F].unsqueeze(1).to_broadcast([128, 11, F]),
                    in1=win(ee, 11, 2),
                    op=ALU.mult,
                )
                # ---- W1 pass -> H, relu evict ----
                for c0, cn in CHUNKS:
                    nc.tensor.matmul(
                        out=hps[:, c0 : c0 + cn],
                        lhsT=ww1_sb[:],
                        rhs=pw[:, c0 : c0 + cn],
                        start=True,
                        stop=True,
                    )
                rh = rh_pool.tile([128, NPC], bf16, tag="rh")
                nc.scalar.activation(
                    out=rh[:], in_=hps[:, 0:NPC], func=AF.Relu, bias=b1v_sb[:]
                )
                # ---- W2 (scores) + Wf (M) accumulation at sample slots ----
                for c0, cn in CHUNKS:
                    nc.tensor.matmul(
                        out=scps[32 * b : 32 * b + 32, c0 : c0 + cn],
                        lhsT=w2b_sb[:, 32 * r : 32 * r + 32],
                        rhs=rh[:, c0 : c0 + cn],
                        tile_position=(0, 32 * b),
                        start=(r == 0),
                        stop=(r == 15),
                        skip_group_check=True,
                    )
                    nc.tensor.matmul(
                        out=mvps[32 * b : 32 * b + 32, c0 : c0 + cn],
                        lhsT=wfb_sb[:, 32 * r : 32 * r + 32],
                        rhs=pw[:, c0 : c0 + cn],
                        tile_position=(0, 32 * b),
                        start=(r == 0),
                        stop=(r == 15),
                        skip_group_check=True,
                    )
                # linear-term sums: after the scores chunk MMs so their
                # start=True bank-region clear cannot wipe these values
                nc.tensor.matmul(
                    out=linps[32 * b : 32 * b + 32, 0:1],
                    lhsT=ones_sb[:, 32 * r : 32 * r + 32],
                    rhs=gr[:, K : K + 1],
                    tile_position=(0, 32 * b),
                    start=(r == 0),
                    stop=(r == 15),
                    skip_group_check=True,
                )
            nc.vector.tensor_copy(out=linsum[:], in_=linps[:])
            # evict accumulators: exp(scores) and M values
            nc.scalar.activation(out=exps[:], in_=scps[:, 0:NPC], func=AF.Exp)
            nc.vector.tensor_copy(out=mvals[:], in_=mvps[:, 0:NPC])

            # ------------- backend: softmax-weighted reduction -----------
            prods = big_pool.tile([128, NPC], f32, tag="prods")
            nc.vector.tensor_tensor(
                out=prods[:], in0=exps[:], in1=mvals[:], op=ALU.mult
            )
            # direct row sums over all 21 stored diagonals
            ap_sum = small_pool.tile([128, F], f32, tag="ap_sum")
            nc.vector.tensor_reduce(
                out=ap_sum[:], in_=winT(prods, NDIAG), axis=AX.X, op=ALU.add,
                apply_transpose=False,
            )
            ae_sum = small_pool.tile([128, F], f32, tag="ae_sum")
            nc.vector.tensor_reduce(
                out=ae_sum[:], in_=winT(exps, NDIAG), axis=AX.X, op=ALU.add,
                apply_transpose=False,
            )
            # mirrored contributions: for d=1..19, row (i-d)%40 also sees this
            # pair; build shifted copies then reduce over the 19 diagonals
            shp = big_pool.tile([128, 19 * F], f32, tag="shp")
            she = big_pool.tile([128, 19 * F], f32, tag="she")
            q = 0
            for tm, d in [(t, 2 * t + 1) for t in range(10)] + [
                (10 + t, 2 * t) for t in range(1, 10)
            ]:
                base = tm * F
                for srctile, dsttile in ((prods, shp), (exps, she)):
                    nc.vector.tensor_copy(
                        out=dsttile[:, q * F + d : q * F + F],
                        in_=srctile[:, base : base + F - d],
                    )
                    nc.vector.tensor_copy(
                        out=dsttile[:, q * F : q * F + d],
                        in_=srctile[:, base + F - d : base + F],
                    )
                q += 1
            bp_sum = small_pool.tile([128, F], f32, tag="bp_sum")
            nc.vector.tensor_reduce(
                out=bp_sum[:], in_=winT(shp, 19), axis=AX.X, op=ALU.add,
                apply_transpose=False,
            )
            be_sum = small_pool.tile([128, F], f32, tag="be_sum")
            nc.vector.tensor_reduce(
                out=be_sum[:], in_=winT(she, 19), axis=AX.X, op=ALU.add,
                apply_transpose=False,
            )
            r40 = small_pool.tile([128, F], f32, tag="r40")
            nc.vector.tensor_tensor(out=r40[:], in0=ap_sum[:], in1=bp_sum[:], op=ALU.add)
            den = small_pool.tile([128, F], f32, tag="den")
            nc.vector.tensor_tensor(out=den[:], in0=ae_sum[:], in1=be_sum[:], op=ALU.add)
            rden = small_pool.tile([128, F], f32, tag="rden")
            nc.vector.reciprocal(out=rden[:], in_=den[:])
            c40 = small_pool.tile([128, F], f32, tag="c40")
            nc.vector.tensor_tensor(out=c40[:], in0=r40[:], in1=rden[:], op=ALU.mult)
            intr = small_pool.tile([128, 1], f32, tag="intr")
            nc.vector.tensor_reduce(out=intr[:], in_=c40[:], axis=AX.X, op=ALU.add)
            logit = small_pool.tile([128, 1], f32, tag="logit")
            nc.vector.tensor_tensor(
                out=logit[:], in0=intr[:], in1=linsum[:], op=ALU.add
            )
            # sigmoid(x + bf) = 1 / (1 + exp(-x - bf)); bfn holds -bf
            enl = small_pool.tile([128, 1], f32, tag="enl")
            nc.scalar.activation(
                out=enl[:], in_=logit[:], func=AF.Exp, bias=bfn_sb[:], scale=-1.0
            )
            onep = small_pool.tile([128, 1], f32, tag="onep")
            nc.vector.tensor_scalar_add(out=onep[:], in0=enl[:], scalar1=1.0)
            sig = small_pool.tile([128, 1], f32, tag="sig")
            nc.vector.reciprocal(out=sig[:], in_=onep[:])
            nc.sync.dma_start(
                out=out_p[m * MS : (m + 1) * MS].unsqueeze(1), in_=sig[:]
            )

    nc.compile()
    return nc


def _prep_in_maps(x, lin_table, V_table, W1, b1, W2, b2, Wf, bf):
    # b2 is dropped exactly (softmax shift invariance).
    x = np.asarray(x).astype(np.int32)
    V_table = np.asarray(V_table, dtype=np.float32)
    lin_table = np.asarray(lin_table, dtype=np.float32).reshape(V, 1)
    W1 = np.asarray(W1, dtype=np.float32)
    W2 = np.asarray(W2, dtype=np.float32).reshape(A, 1)
    Wf = np.asarray(Wf, dtype=np.float32).reshape(K, 1)
    b1 = np.asarray(b1, dtype=np.float32).reshape(A)
    bf = np.float32(np.asarray(bf).reshape(-1)[0])

    # augmented table: [V | lin | pad]
    va = np.zeros((V, VROW), dtype=np.float32)
    va[:, 0:K] = V_table
    va[:, K] = lin_table[:, 0]

    ww1 = np.zeros((128, 128), dtype=np.float32)
    ww1[0:K, 0:A] = W1
    ww1[K:128, A:128] = W1
    # 16 stationary variants: variant r has only columns 2r, 2r+1 nonzero
    w2b = np.zeros((128, 512), dtype=np.float32)
    wfb = np.zeros((128, 512), dtype=np.float32)
    onesb = np.zeros((2 * F, 512), dtype=np.float32)
    for r in range(16):
        w2b[0:A, 32 * r + 2 * r] = W2[:, 0]
        w2b[A:128, 32 * r + 2 * r + 1] = W2[:, 0]
        wfb[0:K, 32 * r + 2 * r] = Wf[:, 0]
        wfb[K:128, 32 * r + 2 * r + 1] = Wf[:, 0]
        onesb[0:F, 32 * r + 2 * r] = 1.0
        onesb[F : 2 * F, 32 * r + 2 * r + 1] = 1.0
    b1v = np.concatenate([b1, b1]).reshape(128, 1).astype(np.float32)
    bfn = np.full((128, 1), -bf, dtype=np.float32)
    ident = np.eye(128, dtype=np.float32)

    xs = x.reshape(NCORES, BL, F)
    in_maps = []
    for c in range(NCORES):
        xc = xs[c]  # [256, 40]
        # xT2[p, gg]: row index for partition p of group gg's gather
        # p in [0,40): field p of sample 2*gg ; p in [40,80): sample 2*gg+1
        xT2 = np.empty((2 * F, NMACRO * NG), dtype=np.int32)
        pairs = xc.reshape(NMACRO * NG, 2, F)  # [128 groups, 2, 40]
        xT2[0:F, :] = pairs[:, 0, :].T
        xT2[F : 2 * F, :] = pairs[:, 1, :].T
        in_maps.append(
            {
                "xT2": xT2,
                "va": va,
                "ww1": ww1,
                "w2b": w2b,
                "wfb": wfb,
                "onesb": onesb,
                "b1v": b1v,
                "bfn": bfn,
                "ident": ident,
            }
        )
    return in_maps


def _get_nc():
    if "nc" not in _CACHE:
        _CACHE["nc"] = _build_program()
    return _CACHE["nc"]


def _enable_ldw_opt():
    # walrus ships with --enable-ldw-opt=false hardcoded; LDWEIGHTS then never
    # overlaps matmuls and costs ~115ns x 9 per group. Rewrite the flag.
    from concourse import bass_utils
    if getattr(bass_utils, "_ldw_patched", False):
        return
    orig = bass_utils.run_command

    def patched(cmd, *a, **kw):
        cmd = [
            c.replace("--enable-ldw-opt=false", "--enable-ldw-opt=true")
            if isinstance(c, str)
            else c
            for c in cmd
        ]
        return orig(cmd, *a, **kw)

    bass_utils.run_command = patched
    bass_utils._ldw_patched = True


def kernel(**inputs):
    from concourse.bass_utils import run_bass_kernel_spmd


    nc = _get_nc()
    in_maps = _prep_in_maps(**inputs)
    res = run_bass_kernel_spmd(nc, in_maps, core_ids=list(range(NCORES)))
    out = np.concatenate([res.results[c]["out"] for c in range(NCORES)])
    return out.astype(np.float32)

